# revision 14
# baseline (speedup 1.0000x reference)
"""Bass/Tile TRN2 kernel for nn_DecoderGroupedQueryHeadAttentionAlibi.

Sharding (8 cores): core = (b, g) with b = core//2 in [0,4) (batch),
g = core%2 (head parity). Slot i on core g computes head 2i+g; slot pairs
(2i, 2i+1) use kv heads {g, g+2}. Host sums the two parity partials + bproj.

Per-core device program (scoresT layout: [s_partitions, t_free]):
  - bf16 projections/scores, interleaved into the first processed slots'
    attention loops (psum borrowed from the shared psS pool); dense slots
    (ne=4) processed FIRST since their ACT load best hides the extra
    Tensor work.
  - per (slot, s-tile): scoresT psum [128,1024] -> ACT exp (alibi linear
    bias folded into the per-partition activation bias) -> DVE multipliers
    (Toeplitz table broadcast-loaded from single DRAM rows) -> attn@v (bf16)
    accumulated in psum [65,2048]; row 64 (ones column in v) is the softmax
    denominator.
  - per slot PAIR: denominator gather/reciprocal/broadcast + outT normalize
    (DVE/DMA only), overlapped with later slots; a lean monolithic output
    projection (bf16 out DMA) forms the tail.

Alibi: bias = min(a_h*(s-t), 0) (tril overwrites the causal mask in the
torch reference; future tokens attend with bias 0), so
P = exp(score/8) * min(exp(a*(s-t)), 1) which factors into a per-partition
ACT bias exp(a*(s_in-127)) and Toeplitz row multipliers per region.
"""

import math
import numpy as np

# ---- problem constants (hardcoded; kernel.py must be self-contained) ----
B, T, C = 4, 2048, 1024
N_HEAD, N_KV_HEAD, HEAD_DIM = 16, 4, 64
NH = 8            # heads per core
ST = T // 128     # 16 s-tiles
NCH = T // 512    # 4 t-chunks
KCT = C // 128    # 8 contraction tiles of 128
WREP_W = 2048     # Toeplitz table width: index = t - 128*j is always < 2048
CUT_MARGIN = 13.0  # exp(-13) ~ 2.3e-6: dropped mass is ~1e-5 of denom

_START = 2.0 ** (-2.0 ** (-(math.log2(N_HEAD) - 3.0)))  # 0.7071...


def _head_of_slot(i: int, g: int) -> int:
    return 2 * i + g


def _a_of_head(h: int) -> float:
    return (_START ** (h + 1)) / math.sqrt(HEAD_DIM)


# Loop bounds must be identical on every core (SPMD): use the widest cutoff
# over g for each head slot (g=1 heads have smaller slopes -> wider bands).
_CUTOFF = [CUT_MARGIN / min(_a_of_head(_head_of_slot(i, 0)),
                            _a_of_head(_head_of_slot(i, 1)))
           for i in range(NH)]
_N_EFF = [[min(NCH, int((128 * j + _CUTOFF[i]) // 512) + 1)
           for j in range(ST)] for i in range(NH)]
_J_FIRST = [[min(j for j in range(ST) if _N_EFF[i][j] > tcn)
             for tcn in range(NCH)] for i in range(NH)]

# slot processing order: dense slots first (their ACT load hides the
# interleaved projection matmuls), pairs contiguous.
_PORDER = [6, 7, 4, 5, 2, 3, 0, 1]

_NC_CACHE = {}


def _split_multiwait(nc, mybir, max_waits=1):
    """walrus in this env encodes at most one sync-wait per instruction;
    split extras onto same-engine NoOps emitted just before."""
    for f in nc.m.functions:
        for bb in f.blocks:
            new = []
            for ins in bb.instructions:
                si = ins.sync_info
                conds = list(si.on_wait) if si is not None else []
                if len(conds) > max_waits:
                    for cond in conds[:-max_waits]:
                        n = mybir.InstNoOp(
                            name=nc.get_next_instruction_name(), ins=[], outs=[])
                        n.engine = ins.engine
                        n.sync_info = mybir.SyncInfo(on_wait=[cond], on_update=[])
                        new.append(n)
                    si.on_wait = conds[-max_waits:]
                new.append(ins)
            bb.instructions = new


def _build_nc(split=True):
    key = "nc" if split else "nc_nosplit"
    if key in _NC_CACHE:
        return _NC_CACHE[key]
    import concourse.bass as bass
    import concourse.tile as tile
    from concourse import mybir

    f32 = mybir.dt.float32
    bf16 = mybir.dt.bfloat16
    f8 = mybir.dt.float8e4
    AF = mybir.ActivationFunctionType
    MUL = mybir.AluOpType.mult
    ADD = mybir.AluOpType.add
    MIN = mybir.AluOpType.min
    DR = mybir.MatmulPerfMode.DoubleRow

    nc = bass.Bass()

    xT_d = nc.dram_tensor("xT", [C, T], bf16, kind="ExternalInput")
    wq_d = nc.dram_tensor("wqT", [C, NH * 64], bf16, kind="ExternalInput")
    wk_d = nc.dram_tensor("wkT", [C, 128], bf16, kind="ExternalInput")
    wv_d = nc.dram_tensor("wvT", [C, 128], bf16, kind="ExternalInput")
    wp_d = nc.dram_tensor("wpT", [NH * 64, C], bf16, kind="ExternalInput")
    wrep1_d = nc.dram_tensor("wrep1", [NH, WREP_W], bf16, kind="ExternalInput")
    u_d = nc.dram_tensor("usb", [128, NH], f32, kind="ExternalInput")
    bias_d = nc.dram_tensor("biassb", [128, NH], f32, kind="ExternalInput")
    out_d = nc.dram_tensor("out", [T, C], bf16, kind="ExternalOutput")

    xT_r = xT_d.rearrange("(k p) t -> p k t", p=128)
    wq_r = wq_d.rearrange("(k p) e -> p k e", p=128)

    with tile.TileContext(nc) as tc:
        with (
            tc.tile_pool(name="const", bufs=1) as const,
        ):
            # ---- persistent tiles ----
            wp = const.tile([128, 4, C], bf16)
            wrep = const.tile([128, NH, WREP_W], bf16)
            usb = const.tile([128, NH], f32)
            biassb = const.tile([128, NH], f32)

            kRep = const.tile([128, 2, T], bf16)     # kv on both halves
            qRep = const.tile([128, NH, T], bf16)    # slot i on both halves
            v_sb = const.tile([128, ST, 130], bf16)  # [s, j, (v_kv0|1|v_kv1|1)]
            outT = const.tile([128, 4, T], bf16)     # [(2 slots d), pair, t]
            dstack = const.tile([128, 128], bf16)    # [(slot,tt), t_in] denom
            rstb = const.tile([128, 128], bf16)      # 1/denom, bf16

            with (
                tc.tile_pool(name="ph1", bufs=1) as ph1,
                tc.tile_pool(name="work", bufs=3) as work,
                tc.tile_pool(name="ebuf", bufs=3) as ebufp,
                tc.tile_pool(name="dstgp", bufs=1) as dstgp,
                tc.tile_pool(name="dramd", bufs=1, space="DRAM") as dramd,
            ):
                xT = ph1.tile([128, KCT, T], bf16)
                wk = ph1.tile([128, KCT, 128], bf16)
                wq = ph1.tile([128, KCT, NH * 64], bf16)
                wv = ph1.tile([128, KCT, 128], bf16)
                ddrow = dramd.tile([NH, T], bf16)
                rdram = dramd.tile([NH, T], bf16)
                rd3 = rdram.rearrange("i (a b) -> i a b", b=128)

                # ---- DMA loads, ordered for the critical path ----
                nc.sync.dma_start(out=wk, in_=wk_d.rearrange(
                    "(k p) e -> p k e", p=128))
                qs = [nc.sync, nc.scalar, nc.gpsimd]
                for kc in range(KCT):
                    qs[kc % 3].dma_start(out=xT[:, kc, :], in_=xT_r[:, kc, :])
                for kc in range(KCT):
                    nc.scalar.dma_start(out=wq[:, kc, :], in_=wq_r[:, kc, :])
                nc.gpsimd.dma_start(out=usb, in_=u_d[:])
                nc.gpsimd.dma_start(out=biassb, in_=bias_d[:])
                nc.gpsimd.dma_start(out=wv, in_=wv_d.rearrange(
                    "(k p) e -> p k e", p=128))
                # Toeplitz rows: broadcast one DRAM row to 128 partitions.
                for i in range(NH):
                    src = wrep1_d[i:i + 1, :]
                    src = bass.AP(tensor=src.tensor, offset=src.offset,
                                  ap=[[0, 128]] + list(src.ap)[1:])
                    nc.gpsimd.dma_start(out=wrep[:, i, :], in_=src)
                nc.gpsimd.dma_start(out=wp, in_=wp_d.rearrange(
                    "(k p) e -> p k e", p=128))
                # ones columns of v (disjoint from the v-proj copies)
                nc.vector.memset(v_sb[:, :, 64], 1.0)
                nc.vector.memset(v_sb[:, :, 129], 1.0)

                # ---- attention with interleaved projections/out-proj ----
                with (
                    tc.tile_pool(name="psA", bufs=1, space="PSUM") as psA,
                    tc.tile_pool(name="psS", bufs=2, space="PSUM") as psS,
                ):
                    def k_proj_chunk(sc):
                        """kRep cols [512sc,512sc+512): kv half0 rows 0:64,
                        half1 rows 64:128, then per-chunk duplication."""
                        ps = psS.tile([128, 512], f32, tag="S")
                        for kc in range(KCT):
                            nc.tensor.matmul(
                                ps, lhsT=wk[:, kc, :],
                                rhs=xT[:, kc, 512 * sc:512 * (sc + 1)],
                                start=(kc == 0), stop=(kc == KCT - 1))
                        sl = slice(512 * sc, 512 * (sc + 1))
                        nc.vector.tensor_copy(kRep[0:64, 0, sl], ps[0:64, :])
                        nc.vector.tensor_copy(kRep[64:128, 1, sl],
                                              ps[64:128, :])
                        nc.sync.dma_start(out=kRep[64:128, 0, sl],
                                          in_=kRep[0:64, 0, sl])
                        nc.sync.dma_start(out=kRep[0:64, 1, sl],
                                          in_=kRep[64:128, 1, sl])

                    def q_proj_block(p, tcn):
                        """qRep slots 2p/2p+1, cols [512tcn, 512tcn+512)."""
                        ps = psS.tile([128, 512], f32, tag="S")
                        for kc in range(KCT):
                            nc.tensor.matmul(
                                ps, lhsT=wq[:, kc, 128 * p:128 * (p + 1)],
                                rhs=xT[:, kc, 512 * tcn:512 * (tcn + 1)],
                                start=(kc == 0), stop=(kc == KCT - 1))
                        sl = slice(512 * tcn, 512 * (tcn + 1))
                        nc.vector.tensor_copy(qRep[0:64, 2 * p, sl],
                                              ps[0:64, :])
                        nc.vector.tensor_copy(qRep[64:128, 2 * p + 1, sl],
                                              ps[64:128, :])
                        nc.sync.dma_start(out=qRep[64:128, 2 * p, sl],
                                          in_=qRep[0:64, 2 * p, sl])
                        nc.sync.dma_start(out=qRep[0:64, 2 * p + 1, sl],
                                          in_=qRep[64:128, 2 * p + 1, sl])

                    def v_proj_block(st):
                        ps = psS.tile([128, 512], f32, tag="S")
                        for kc in range(KCT):
                            nc.tensor.matmul(
                                ps[:, 0:128],
                                lhsT=xT[:, kc, 128 * st:128 * (st + 1)],
                                rhs=wv[:, kc, :],
                                start=(kc == 0), stop=(kc == KCT - 1))
                        nc.vector.tensor_copy(v_sb[:, st, 0:64], ps[:, 0:64])
                        nc.vector.tensor_copy(v_sb[:, st, 65:129],
                                              ps[:, 64:128])

                    def pair_chain(p):
                        """denominator reciprocal + outT normalize, pair p."""
                        rows = slice(32 * p, 32 * (p + 1))
                        dstf = work.tile([32, 128], f32, tag="dstf")
                        nc.vector.tensor_copy(dstf, dstack[rows, :])
                        rstf = work.tile([32, 128], f32, tag="rstf")
                        nc.vector.reciprocal(rstf, dstf)
                        nc.vector.tensor_copy(rstb[rows, :], rstf)
                        for hh in range(2):
                            ii = 2 * p + hh
                            nc.sync.dma_start(
                                out=rd3[ii],
                                in_=rstb[16 * ii:16 * (ii + 1), :])
                        rrep = work.tile([128, T], bf16, tag="rrep", bufs=1)
                        for hh in range(2):
                            ii = 2 * p + hh
                            src = rdram[ii:ii + 1, :]
                            src = bass.AP(tensor=src.tensor, offset=src.offset,
                                          ap=[[0, 64]] + list(src.ap)[1:])
                            nc.sync.dma_start(
                                out=rrep[64 * hh:64 * hh + 64, :], in_=src)
                        nc.vector.tensor_tensor(outT[:, p, :], outT[:, p, :],
                                                rrep, MUL)

                    # schedule interleaved work by processing POSITION
                    prework = [[[] for _ in range(ST)] for _ in range(NH)]
                    first_slot = _PORDER[0]
                    fq = first_slot // 2
                    # upfront: k chunk 0 + the first slot's full q pair
                    prework[0][0].append(lambda: k_proj_chunk(0))
                    prework[0][0].append(lambda: q_proj_block(fq, 0))
                    for tcn in (1, 2, 3):
                        jn = _J_FIRST[first_slot][tcn]
                        prework[0][max(0, jn - 1) if jn > 0 else 0].insert(
                            0, lambda tcn=tcn: q_proj_block(fq, tcn))
                    for sc in (1, 2, 3):
                        prework[0][4 * sc - 3].append(
                            lambda sc=sc: k_proj_chunk(sc))
                    for st in range(ST):
                        prework[0][st].append(lambda st=st: v_proj_block(st))
                    # remaining q pairs at positions 1..3, j = 2,5,8,11;
                    # pair for the slots at positions 2m,2m+1 must be emitted
                    # by position 2m-1 -> emission order follows _PORDER.
                    qpairs = [_PORDER[2 * m] // 2 for m in range(1, 4)]
                    for pos, p in enumerate(qpairs):
                        for tcn in range(NCH):
                            prework[1 + pos][2 + 3 * tcn].append(
                                lambda p=p, tcn=tcn: q_proj_block(p, tcn))

                    for pos in range(NH):
                        i = _PORDER[pos]
                        p, half = i // 2, i % 2
                        pa = psA.tile([65, T], f32, tag="pa")
                        # diag mult min(exp(-a(t_in-127)), exp(a(127-s_in)))
                        dmin = work.tile([128, 128], bf16, tag="dmin")
                        nc.vector.tensor_scalar(dmin, wrep[:, i, 0:128],
                                                usb[:, i:i + 1], None, MIN)
                        for j in range(ST):
                            for fn in prework[pos][j]:
                                fn()
                            ne = _N_EFF[i][j]
                            W = 512 * ne
                            E = ebufp.tile([128, T], bf16, tag="E")
                            for sh in range(2):
                                c0, c1 = 2 * sh, min(ne, 2 * sh + 2)
                                if c0 >= c1:
                                    continue
                                S = psS.tile([128, 1024], f32, tag="S")
                                for tcn in range(c0, c1):
                                    rh = 64 * (tcn % 2)
                                    o = 512 * (tcn - c0)
                                    nc.tensor.matmul(
                                        S[:, o:o + 512],
                                        lhsT=kRep[rh:rh + 64, half,
                                                  128 * j:128 * (j + 1)],
                                        rhs=qRep[rh:rh + 64, i,
                                                 512 * tcn:512 * (tcn + 1)],
                                        start=True, stop=True)
                                wv_ = 512 * (c1 - c0)
                                nc.scalar.activation(
                                    E[:, 1024 * sh:1024 * sh + wv_],
                                    S[:, :wv_], AF.Exp,
                                    bias=biassb[:, i:i + 1], scale=0.125)
                            lo = 128 * j       # t < lo: future region (mult u)
                            hi = 128 * (j + 1)  # t >= hi: past (Toeplitz)
                            if lo > 0:
                                nc.vector.tensor_scalar(
                                    E[:, :lo], E[:, :lo],
                                    usb[:, i:i + 1], None, MUL)
                            nc.vector.tensor_tensor(E[:, lo:hi], E[:, lo:hi],
                                                    dmin, MUL)
                            if W > hi:
                                nc.vector.tensor_tensor(
                                    E[:, hi:W], E[:, hi:W],
                                    wrep[:, i, 128:128 + (W - hi)], MUL)
                            for tcn in range(ne):
                                nc.tensor.matmul(
                                    pa[:, 512 * tcn:512 * (tcn + 1)],
                                    lhsT=v_sb[:, j, 65 * half:65 * half + 65],
                                    rhs=E[:, 512 * tcn:512 * (tcn + 1)],
                                    start=(j == _J_FIRST[i][tcn]),
                                    stop=(j == ST - 1),
                                    skip_group_check=True)
                        # copy-out: rows 0:64 -> outT half; row 64 -> denom
                        st65 = dstgp.tile([65, T], bf16, tag="st65")
                        nc.vector.tensor_copy(st65, pa[0:65, :])
                        nc.sync.dma_start(
                            out=outT[64 * half:64 * half + 64, p, :],
                            in_=st65[0:64, :])
                        nc.sync.dma_start(out=ddrow[i:i + 1, :],
                                          in_=st65[64:65, :])
                        nc.sync.dma_start(
                            out=dstack[16 * i:16 * (i + 1), :],
                            in_=ddrow[i].rearrange("(a b) -> a b", b=128))
                        if pos % 2 == 1:
                            pair_chain(p)

                    # ---- tail: output projection + output DMA ----
                    for tt in range(ST):
                        osb = work.tile([128, C], bf16, tag="osb", bufs=2)
                        for ec in range(2):
                            ps = psS.tile([128, 512], f32, tag="S")
                            for kt in range(4):
                                nc.tensor.matmul(
                                    ps,
                                    lhsT=outT[:, kt, 128 * tt:128 * (tt + 1)],
                                    rhs=wp[:, kt, 512 * ec:512 * (ec + 1)],
                                    start=(kt == 0), stop=(kt == 3))
                            nc.vector.tensor_copy(
                                osb[:, 512 * ec:512 * (ec + 1)], ps)
                        eng = nc.sync if tt % 2 == 0 else nc.scalar
                        eng.dma_start(out=out_d[128 * tt:128 * (tt + 1), :],
                                      in_=osb)

    if split:
        _split_multiwait(nc, mybir)
    _NC_CACHE[key] = nc
    return nc


def _prep_core_inputs(x, Wq, Wkv, Wproj, b, g):
    import ml_dtypes
    bf = ml_dtypes.bfloat16
    heads = [_head_of_slot(i, g) for i in range(NH)]
    xT = np.ascontiguousarray(x[b].T).astype(bf)                      # [C, T]
    wq_cols = np.concatenate([Wq[64 * h:64 * (h + 1)] for h in heads], axis=0)
    wqT = np.ascontiguousarray(wq_cols.T).astype(bf)                  # [C, 512]
    # kv heads used by parity-g slots: half 0 -> kv g, half 1 -> kv g+2
    kv = [g, g + 2]
    wk_rows = np.concatenate([Wkv[64 * kvh:64 * (kvh + 1)] for kvh in kv])
    wv_rows = np.concatenate([Wkv[256 + 64 * kvh:256 + 64 * (kvh + 1)]
                              for kvh in kv])
    wkT = np.ascontiguousarray(wk_rows.T).astype(bf)
    wvT = np.ascontiguousarray(wv_rows.T).astype(bf)
    cols = np.concatenate([np.arange(64 * h, 64 * (h + 1)) for h in heads])
    wpT = np.ascontiguousarray(Wproj[:, cols].T).astype(bf)           # [512, C]

    s_in = np.arange(128, dtype=np.float64)
    wrep1 = np.empty((NH, WREP_W), dtype=bf)
    u = np.empty((128, NH), dtype=np.float32)
    bias = np.empty((128, NH), dtype=np.float32)
    idx = np.arange(WREP_W, dtype=np.float64)
    for i, h in enumerate(heads):
        a = _a_of_head(h)
        wrep1[i] = np.exp(-a * (idx - 127.0)).astype(np.float32)
        u[:, i] = np.exp(a * (127.0 - s_in)).astype(np.float32)
        bias[:, i] = (a * (s_in - 127.0)).astype(np.float32)
    return {"xT": xT, "wqT": wqT, "wkT": wkT, "wvT": wvT, "wpT": wpT,
            "wrep1": wrep1, "usb": u, "biassb": bias}


def kernel(x, Wq, Wkv, Wproj, bproj):
    from concourse.bass_utils import run_bass_kernel_spmd
    x = np.asarray(x, dtype=np.float32)
    Wq = np.asarray(Wq, dtype=np.float32)
    Wkv = np.asarray(Wkv, dtype=np.float32)
    Wproj = np.asarray(Wproj, dtype=np.float32)
    bproj = np.asarray(bproj, dtype=np.float32)

    nc = _build_nc()
    in_maps = [_prep_core_inputs(x, Wq, Wkv, Wproj, c // 2, c % 2)
               for c in range(8)]
    res = run_bass_kernel_spmd(nc, in_maps, core_ids=list(range(8)))
    out = np.zeros((B, T, C), dtype=np.float32)
    for c in range(8):
        out[c // 2] += np.asarray(res.results[c]["out"], dtype=np.float32)
    out += bproj[None, None, :]
    return out


# revision 16
# speedup vs baseline: 1.0059x; 1.0059x over previous
"""Bass/Tile TRN2 kernel for nn_DecoderGroupedQueryHeadAttentionAlibi.

Sharding (8 cores): core = (b, g) with b = core//2 in [0,4) (batch),
g = core%2 (head parity). Slot i on core g computes head 2i+g; slot pairs
(2i, 2i+1) use kv heads {g, g+2}. Host sums the two parity partials + bproj.

Per-core device program (scoresT layout: [s_partitions, t_free]):
  - bf16 projections/scores, interleaved into the first processed slots'
    attention loops (psum borrowed from the shared psS pool); dense slots
    (ne=4) processed FIRST since their ACT load best hides the extra
    Tensor work.
  - per (slot, s-tile): scoresT psum [128,1024] -> ACT exp (alibi linear
    bias folded into the per-partition activation bias) -> DVE multipliers
    (Toeplitz table broadcast-loaded from single DRAM rows) -> attn@v (bf16)
    accumulated in psum [65,2048]; row 64 (ones column in v) is the softmax
    denominator.
  - per slot PAIR: denominator gather/reciprocal/broadcast + outT normalize
    (DVE/DMA only), overlapped with later slots; a lean monolithic output
    projection (bf16 out DMA) forms the tail.

Alibi: bias = min(a_h*(s-t), 0) (tril overwrites the causal mask in the
torch reference; future tokens attend with bias 0), so
P = exp(score/8) * min(exp(a*(s-t)), 1) which factors into a per-partition
ACT bias exp(a*(s_in-127)) and Toeplitz row multipliers per region.
"""

import math
import numpy as np

# ---- problem constants (hardcoded; kernel.py must be self-contained) ----
B, T, C = 4, 2048, 1024
N_HEAD, N_KV_HEAD, HEAD_DIM = 16, 4, 64
NH = 8            # heads per core
ST = T // 128     # 16 s-tiles
NCH = T // 512    # 4 t-chunks
KCT = C // 128    # 8 contraction tiles of 128
WREP_W = 2048     # Toeplitz table width: index = t - 128*j is always < 2048
CUT_MARGIN = 13.0  # exp(-13) ~ 2.3e-6: dropped mass is ~1e-5 of denom

_START = 2.0 ** (-2.0 ** (-(math.log2(N_HEAD) - 3.0)))  # 0.7071...


def _head_of_slot(i: int, g: int) -> int:
    return 2 * i + g


def _a_of_head(h: int) -> float:
    return (_START ** (h + 1)) / math.sqrt(HEAD_DIM)


# Loop bounds must be identical on every core (SPMD): use the widest cutoff
# over g for each head slot (g=1 heads have smaller slopes -> wider bands).
_CUTOFF = [CUT_MARGIN / min(_a_of_head(_head_of_slot(i, 0)),
                            _a_of_head(_head_of_slot(i, 1)))
           for i in range(NH)]
_N_EFF = [[min(NCH, int((128 * j + _CUTOFF[i]) // 512) + 1)
           for j in range(ST)] for i in range(NH)]
_J_FIRST = [[min(j for j in range(ST) if _N_EFF[i][j] > tcn)
             for tcn in range(NCH)] for i in range(NH)]

# slot processing order: dense slots first (their ACT load hides the
# interleaved projection matmuls), pairs contiguous.
_PORDER = [2, 3, 6, 7, 4, 5, 0, 1]

_NC_CACHE = {}


def _split_multiwait(nc, mybir, max_waits=1):
    """walrus in this env encodes at most one sync-wait per instruction;
    split extras onto same-engine NoOps emitted just before."""
    for f in nc.m.functions:
        for bb in f.blocks:
            new = []
            for ins in bb.instructions:
                si = ins.sync_info
                conds = list(si.on_wait) if si is not None else []
                if len(conds) > max_waits:
                    for cond in conds[:-max_waits]:
                        n = mybir.InstNoOp(
                            name=nc.get_next_instruction_name(), ins=[], outs=[])
                        n.engine = ins.engine
                        n.sync_info = mybir.SyncInfo(on_wait=[cond], on_update=[])
                        new.append(n)
                    si.on_wait = conds[-max_waits:]
                new.append(ins)
            bb.instructions = new


def _build_nc(split=True):
    key = "nc" if split else "nc_nosplit"
    if key in _NC_CACHE:
        return _NC_CACHE[key]
    import concourse.bass as bass
    import concourse.tile as tile
    from concourse import mybir

    f32 = mybir.dt.float32
    bf16 = mybir.dt.bfloat16
    f8 = mybir.dt.float8e4
    AF = mybir.ActivationFunctionType
    MUL = mybir.AluOpType.mult
    ADD = mybir.AluOpType.add
    MIN = mybir.AluOpType.min
    DR = mybir.MatmulPerfMode.DoubleRow

    nc = bass.Bass()

    xT_d = nc.dram_tensor("xT", [C, T], bf16, kind="ExternalInput")
    wq_d = nc.dram_tensor("wqT", [C, NH * 64], bf16, kind="ExternalInput")
    wk_d = nc.dram_tensor("wkT", [C, 128], bf16, kind="ExternalInput")
    wv_d = nc.dram_tensor("wvT", [C, 128], bf16, kind="ExternalInput")
    wp_d = nc.dram_tensor("wpT", [NH * 64, C], bf16, kind="ExternalInput")
    wrep1_d = nc.dram_tensor("wrep1", [NH, WREP_W], bf16, kind="ExternalInput")
    u_d = nc.dram_tensor("usb", [128, NH], f32, kind="ExternalInput")
    bias_d = nc.dram_tensor("biassb", [128, NH], f32, kind="ExternalInput")
    out_d = nc.dram_tensor("out", [T, C], bf16, kind="ExternalOutput")

    xT_r = xT_d.rearrange("(k p) t -> p k t", p=128)
    wq_r = wq_d.rearrange("(k p) e -> p k e", p=128)

    with tile.TileContext(nc) as tc:
        with (
            tc.tile_pool(name="const", bufs=1) as const,
        ):
            # ---- persistent tiles ----
            wp = const.tile([128, 4, C], bf16)
            wrep = const.tile([128, NH, WREP_W], bf16)
            usb = const.tile([128, NH], f32)
            biassb = const.tile([128, NH], f32)

            kRep = const.tile([128, 2, T], bf16)     # kv on both halves
            qRep = const.tile([128, NH, T], bf16)    # slot i on both halves
            v_sb = const.tile([128, ST, 130], bf16)  # [s, j, (v_kv0|1|v_kv1|1)]
            outT = const.tile([128, 4, T], bf16)     # [(2 slots d), pair, t]
            dstack = const.tile([128, 128], bf16)    # [(slot,tt), t_in] denom
            rstb = const.tile([128, 128], bf16)      # 1/denom, bf16

            with (
                tc.tile_pool(name="ph1", bufs=1) as ph1,
                tc.tile_pool(name="work", bufs=3) as work,
                tc.tile_pool(name="ebuf", bufs=3) as ebufp,
                tc.tile_pool(name="dstgp", bufs=1) as dstgp,
                tc.tile_pool(name="dramd", bufs=1, space="DRAM") as dramd,
            ):
                xT = ph1.tile([128, KCT, T], bf16)
                wk = ph1.tile([128, KCT, 128], bf16)
                wq = ph1.tile([128, KCT, NH * 64], bf16)
                wv = ph1.tile([128, KCT, 128], bf16)
                ddrow = dramd.tile([NH, T], bf16)
                rdram = dramd.tile([NH, T], bf16)
                rd3 = rdram.rearrange("i (a b) -> i a b", b=128)

                # ---- DMA loads, ordered for the critical path ----
                nc.sync.dma_start(out=wk, in_=wk_d.rearrange(
                    "(k p) e -> p k e", p=128))
                qs = [nc.sync, nc.scalar, nc.gpsimd]
                for kc in range(KCT):
                    qs[kc % 3].dma_start(out=xT[:, kc, :], in_=xT_r[:, kc, :])
                for kc in range(KCT):
                    nc.scalar.dma_start(out=wq[:, kc, :], in_=wq_r[:, kc, :])
                nc.gpsimd.dma_start(out=usb, in_=u_d[:])
                nc.gpsimd.dma_start(out=biassb, in_=bias_d[:])
                nc.gpsimd.dma_start(out=wv, in_=wv_d.rearrange(
                    "(k p) e -> p k e", p=128))
                # Toeplitz rows: broadcast one DRAM row to 128 partitions.
                for i in range(NH):
                    src = wrep1_d[i:i + 1, :]
                    src = bass.AP(tensor=src.tensor, offset=src.offset,
                                  ap=[[0, 128]] + list(src.ap)[1:])
                    nc.gpsimd.dma_start(out=wrep[:, i, :], in_=src)
                nc.gpsimd.dma_start(out=wp, in_=wp_d.rearrange(
                    "(k p) e -> p k e", p=128))
                # ones columns of v (disjoint from the v-proj copies)
                nc.vector.memset(v_sb[:, :, 64], 1.0)
                nc.vector.memset(v_sb[:, :, 129], 1.0)

                # ---- attention with interleaved projections/out-proj ----
                with (
                    tc.tile_pool(name="psA", bufs=1, space="PSUM") as psA,
                    tc.tile_pool(name="psS", bufs=2, space="PSUM") as psS,
                ):
                    def k_proj_chunk(sc):
                        """kRep cols [512sc,512sc+512): kv half0 rows 0:64,
                        half1 rows 64:128, then per-chunk duplication."""
                        ps = psS.tile([128, 512], f32, tag="S")
                        for kc in range(KCT):
                            nc.tensor.matmul(
                                ps, lhsT=wk[:, kc, :],
                                rhs=xT[:, kc, 512 * sc:512 * (sc + 1)],
                                start=(kc == 0), stop=(kc == KCT - 1))
                        sl = slice(512 * sc, 512 * (sc + 1))
                        nc.vector.tensor_copy(kRep[0:64, 0, sl], ps[0:64, :])
                        nc.vector.tensor_copy(kRep[64:128, 1, sl],
                                              ps[64:128, :])
                        nc.sync.dma_start(out=kRep[64:128, 0, sl],
                                          in_=kRep[0:64, 0, sl])
                        nc.sync.dma_start(out=kRep[0:64, 1, sl],
                                          in_=kRep[64:128, 1, sl])

                    def q_proj_block(p, tcn):
                        """qRep slots 2p/2p+1, cols [512tcn, 512tcn+512)."""
                        ps = psS.tile([128, 512], f32, tag="S")
                        for kc in range(KCT):
                            nc.tensor.matmul(
                                ps, lhsT=wq[:, kc, 128 * p:128 * (p + 1)],
                                rhs=xT[:, kc, 512 * tcn:512 * (tcn + 1)],
                                start=(kc == 0), stop=(kc == KCT - 1))
                        sl = slice(512 * tcn, 512 * (tcn + 1))
                        nc.vector.tensor_copy(qRep[0:64, 2 * p, sl],
                                              ps[0:64, :])
                        nc.vector.tensor_copy(qRep[64:128, 2 * p + 1, sl],
                                              ps[64:128, :])
                        nc.sync.dma_start(out=qRep[64:128, 2 * p, sl],
                                          in_=qRep[0:64, 2 * p, sl])
                        nc.sync.dma_start(out=qRep[0:64, 2 * p + 1, sl],
                                          in_=qRep[64:128, 2 * p + 1, sl])

                    def v_proj_block(st):
                        ps = psS.tile([128, 512], f32, tag="S")
                        for kc in range(KCT):
                            nc.tensor.matmul(
                                ps[:, 0:128],
                                lhsT=xT[:, kc, 128 * st:128 * (st + 1)],
                                rhs=wv[:, kc, :],
                                start=(kc == 0), stop=(kc == KCT - 1))
                        nc.vector.tensor_copy(v_sb[:, st, 0:64], ps[:, 0:64])
                        nc.vector.tensor_copy(v_sb[:, st, 65:129],
                                              ps[:, 64:128])

                    def pair_chain(p):
                        """denominator reciprocal + outT normalize, pair p."""
                        rows = slice(32 * p, 32 * (p + 1))
                        dstf = work.tile([32, 128], f32, tag="dstf")
                        nc.vector.tensor_copy(dstf, dstack[rows, :])
                        rstf = work.tile([32, 128], f32, tag="rstf")
                        nc.vector.reciprocal(rstf, dstf)
                        nc.vector.tensor_copy(rstb[rows, :], rstf)
                        for hh in range(2):
                            ii = 2 * p + hh
                            nc.sync.dma_start(
                                out=rd3[ii],
                                in_=rstb[16 * ii:16 * (ii + 1), :])
                        rrep = work.tile([128, T], bf16, tag="rrep", bufs=1)
                        for hh in range(2):
                            ii = 2 * p + hh
                            src = rdram[ii:ii + 1, :]
                            src = bass.AP(tensor=src.tensor, offset=src.offset,
                                          ap=[[0, 64]] + list(src.ap)[1:])
                            nc.sync.dma_start(
                                out=rrep[64 * hh:64 * hh + 64, :], in_=src)
                        nc.vector.tensor_tensor(outT[:, p, :], outT[:, p, :],
                                                rrep, MUL)

                    # schedule interleaved work by processing POSITION
                    prework = [[[] for _ in range(ST)] for _ in range(NH)]
                    first_slot = _PORDER[0]
                    fq = first_slot // 2
                    # upfront: k chunk 0 + the first slot's full q pair
                    prework[0][0].append(lambda: k_proj_chunk(0))
                    ne0 = _N_EFF[first_slot][0]
                    for tcn in range(ne0):
                        prework[0][0].append(
                            lambda tcn=tcn: q_proj_block(fq, tcn))
                    for tcn in range(ne0, NCH):
                        jn = _J_FIRST[first_slot][tcn]
                        prework[0][max(0, jn - 2)].insert(
                            0, lambda tcn=tcn: q_proj_block(fq, tcn))
                    for sc in (1, 2, 3):
                        prework[0][4 * sc - 3].append(
                            lambda sc=sc: k_proj_chunk(sc))
                    for st in range(ST):
                        prework[0][st].append(lambda st=st: v_proj_block(st))
                    # remaining q pairs at positions 1..3, j = 2,5,8,11;
                    # pair for the slots at positions 2m,2m+1 must be emitted
                    # by position 2m-1 -> emission order follows _PORDER.
                    qpairs = [_PORDER[2 * m] // 2 for m in range(1, 4)]
                    for pos, p in enumerate(qpairs):
                        for tcn in range(NCH):
                            prework[1 + pos][2 + 3 * tcn].append(
                                lambda p=p, tcn=tcn: q_proj_block(p, tcn))

                    for pos in range(NH):
                        i = _PORDER[pos]
                        p, half = i // 2, i % 2
                        pa = psA.tile([65, T], f32, tag="pa")
                        # diag mult min(exp(-a(t_in-127)), exp(a(127-s_in)))
                        dmin = work.tile([128, 128], bf16, tag="dmin")
                        nc.vector.tensor_scalar(dmin, wrep[:, i, 0:128],
                                                usb[:, i:i + 1], None, MIN)
                        for j in range(ST):
                            for fn in prework[pos][j]:
                                fn()
                            ne = _N_EFF[i][j]
                            W = 512 * ne
                            E = ebufp.tile([128, T], bf16, tag="E")
                            for sh in range(2):
                                c0, c1 = 2 * sh, min(ne, 2 * sh + 2)
                                if c0 >= c1:
                                    continue
                                S = psS.tile([128, 1024], f32, tag="S")
                                for tcn in range(c0, c1):
                                    rh = 64 * (tcn % 2)
                                    o = 512 * (tcn - c0)
                                    nc.tensor.matmul(
                                        S[:, o:o + 512],
                                        lhsT=kRep[rh:rh + 64, half,
                                                  128 * j:128 * (j + 1)],
                                        rhs=qRep[rh:rh + 64, i,
                                                 512 * tcn:512 * (tcn + 1)],
                                        start=True, stop=True)
                                wv_ = 512 * (c1 - c0)
                                nc.scalar.activation(
                                    E[:, 1024 * sh:1024 * sh + wv_],
                                    S[:, :wv_], AF.Exp,
                                    bias=biassb[:, i:i + 1], scale=0.125)
                            lo = 128 * j       # t < lo: future region (mult u)
                            hi = 128 * (j + 1)  # t >= hi: past (Toeplitz)
                            if lo > 0:
                                nc.vector.tensor_scalar(
                                    E[:, :lo], E[:, :lo],
                                    usb[:, i:i + 1], None, MUL)
                            nc.vector.tensor_tensor(E[:, lo:hi], E[:, lo:hi],
                                                    dmin, MUL)
                            if W > hi:
                                nc.vector.tensor_tensor(
                                    E[:, hi:W], E[:, hi:W],
                                    wrep[:, i, 128:128 + (W - hi)], MUL)
                            for tcn in range(ne):
                                nc.tensor.matmul(
                                    pa[:, 512 * tcn:512 * (tcn + 1)],
                                    lhsT=v_sb[:, j, 65 * half:65 * half + 65],
                                    rhs=E[:, 512 * tcn:512 * (tcn + 1)],
                                    start=(j == _J_FIRST[i][tcn]),
                                    stop=(j == ST - 1),
                                    skip_group_check=True)
                        # copy-out: rows 0:64 -> outT half; row 64 -> denom
                        st65 = dstgp.tile([65, T], bf16, tag="st65")
                        nc.vector.tensor_copy(st65, pa[0:65, :])
                        nc.sync.dma_start(
                            out=outT[64 * half:64 * half + 64, p, :],
                            in_=st65[0:64, :])
                        nc.sync.dma_start(out=ddrow[i:i + 1, :],
                                          in_=st65[64:65, :])
                        nc.sync.dma_start(
                            out=dstack[16 * i:16 * (i + 1), :],
                            in_=ddrow[i].rearrange("(a b) -> a b", b=128))
                        if pos % 2 == 1:
                            pair_chain(p)

                    # ---- tail: output projection + output DMA ----
                    for tt in range(ST):
                        osb = work.tile([128, C], bf16, tag="osb", bufs=2)
                        for ec in range(2):
                            ps = psS.tile([128, 512], f32, tag="S")
                            for kt in range(4):
                                nc.tensor.matmul(
                                    ps,
                                    lhsT=outT[:, kt, 128 * tt:128 * (tt + 1)],
                                    rhs=wp[:, kt, 512 * ec:512 * (ec + 1)],
                                    start=(kt == 0), stop=(kt == 3))
                            nc.vector.tensor_copy(
                                osb[:, 512 * ec:512 * (ec + 1)], ps)
                        eng = nc.sync if tt % 2 == 0 else nc.scalar
                        eng.dma_start(out=out_d[128 * tt:128 * (tt + 1), :],
                                      in_=osb)

    if split:
        _split_multiwait(nc, mybir)
    _NC_CACHE[key] = nc
    return nc


def _prep_core_inputs(x, Wq, Wkv, Wproj, b, g):
    import ml_dtypes
    bf = ml_dtypes.bfloat16
    heads = [_head_of_slot(i, g) for i in range(NH)]
    xT = np.ascontiguousarray(x[b].T).astype(bf)                      # [C, T]
    wq_cols = np.concatenate([Wq[64 * h:64 * (h + 1)] for h in heads], axis=0)
    wqT = np.ascontiguousarray(wq_cols.T).astype(bf)                  # [C, 512]
    # kv heads used by parity-g slots: half 0 -> kv g, half 1 -> kv g+2
    kv = [g, g + 2]
    wk_rows = np.concatenate([Wkv[64 * kvh:64 * (kvh + 1)] for kvh in kv])
    wv_rows = np.concatenate([Wkv[256 + 64 * kvh:256 + 64 * (kvh + 1)]
                              for kvh in kv])
    wkT = np.ascontiguousarray(wk_rows.T).astype(bf)
    wvT = np.ascontiguousarray(wv_rows.T).astype(bf)
    cols = np.concatenate([np.arange(64 * h, 64 * (h + 1)) for h in heads])
    wpT = np.ascontiguousarray(Wproj[:, cols].T).astype(bf)           # [512, C]

    s_in = np.arange(128, dtype=np.float64)
    wrep1 = np.empty((NH, WREP_W), dtype=bf)
    u = np.empty((128, NH), dtype=np.float32)
    bias = np.empty((128, NH), dtype=np.float32)
    idx = np.arange(WREP_W, dtype=np.float64)
    for i, h in enumerate(heads):
        a = _a_of_head(h)
        wrep1[i] = np.exp(-a * (idx - 127.0)).astype(np.float32)
        u[:, i] = np.exp(a * (127.0 - s_in)).astype(np.float32)
        bias[:, i] = (a * (s_in - 127.0)).astype(np.float32)
    return {"xT": xT, "wqT": wqT, "wkT": wkT, "wvT": wvT, "wpT": wpT,
            "wrep1": wrep1, "usb": u, "biassb": bias}


def kernel(x, Wq, Wkv, Wproj, bproj):
    from concourse.bass_utils import run_bass_kernel_spmd
    x = np.asarray(x, dtype=np.float32)
    Wq = np.asarray(Wq, dtype=np.float32)
    Wkv = np.asarray(Wkv, dtype=np.float32)
    Wproj = np.asarray(Wproj, dtype=np.float32)
    bproj = np.asarray(bproj, dtype=np.float32)

    nc = _build_nc()
    in_maps = [_prep_core_inputs(x, Wq, Wkv, Wproj, c // 2, c % 2)
               for c in range(8)]
    res = run_bass_kernel_spmd(nc, in_maps, core_ids=list(range(8)))
    out = np.zeros((B, T, C), dtype=np.float32)
    for c in range(8):
        out[c // 2] += np.asarray(res.results[c]["out"], dtype=np.float32)
    out += bproj[None, None, :]
    return out


# revision 19
# speedup vs baseline: 1.0096x; 1.0037x over previous
"""Bass/Tile TRN2 kernel for nn_DecoderGroupedQueryHeadAttentionAlibi.

Sharding (8 cores): core = (b, g) with b = core//2 in [0,4) (batch),
g = core%2 (head parity). Slot i on core g computes head 2i+g; slot pairs
(2i, 2i+1) use kv heads {g, g+2}. Host sums the two parity partials + bproj.

Per-core device program (scoresT layout: [s_partitions, t_free]):
  - bf16 projections/scores, interleaved into the first processed slots'
    attention loops (psum borrowed from the shared psS pool); dense slots
    (ne=4) processed FIRST since their ACT load best hides the extra
    Tensor work.
  - per (slot, s-tile): scoresT psum [128,1024] -> ACT exp (alibi linear
    bias folded into the per-partition activation bias) -> DVE multipliers
    (Toeplitz table broadcast-loaded from single DRAM rows) -> attn@v (bf16)
    accumulated in psum [65,2048]; row 64 (ones column in v) is the softmax
    denominator.
  - per slot PAIR: denominator gather/reciprocal/broadcast + outT normalize
    (DVE/DMA only), overlapped with later slots; a lean monolithic output
    projection (bf16 out DMA) forms the tail.

Alibi: bias = min(a_h*(s-t), 0) (tril overwrites the causal mask in the
torch reference; future tokens attend with bias 0), so
P = exp(score/8) * min(exp(a*(s-t)), 1) which factors into a per-partition
ACT bias exp(a*(s_in-127)) and Toeplitz row multipliers per region.
"""

import math
import numpy as np

# ---- problem constants (hardcoded; kernel.py must be self-contained) ----
B, T, C = 4, 2048, 1024
N_HEAD, N_KV_HEAD, HEAD_DIM = 16, 4, 64
NH = 8            # heads per core
ST = T // 128     # 16 s-tiles
NCH = T // 512    # 4 t-chunks
KCT = C // 128    # 8 contraction tiles of 128
WREP_W = 2048     # Toeplitz table width: index = t - 128*j is always < 2048
CUT_MARGIN = 13.0  # exp(-13) ~ 2.3e-6: dropped mass is ~1e-5 of denom

_START = 2.0 ** (-2.0 ** (-(math.log2(N_HEAD) - 3.0)))  # 0.7071...


def _head_of_slot(i: int, g: int) -> int:
    return 2 * i + g


def _a_of_head(h: int) -> float:
    return (_START ** (h + 1)) / math.sqrt(HEAD_DIM)


# Loop bounds must be identical on every core (SPMD): use the widest cutoff
# over g for each head slot (g=1 heads have smaller slopes -> wider bands).
_CUTOFF = [(13.0 if i == 0 else 10.0)
           / min(_a_of_head(_head_of_slot(i, 0)),
                 _a_of_head(_head_of_slot(i, 1)))
           for i in range(NH)]
_N_EFF = [[min(NCH, int((128 * j + _CUTOFF[i]) // 512) + 1)
           for j in range(ST)] for i in range(NH)]
_J_FIRST = [[min(j for j in range(ST) if _N_EFF[i][j] > tcn)
             for tcn in range(NCH)] for i in range(NH)]

# slot processing order: dense slots first (their ACT load hides the
# interleaved projection matmuls), pairs contiguous.
_PORDER = [2, 3, 6, 7, 4, 5, 0, 1]

_NC_CACHE = {}


def _split_multiwait(nc, mybir, max_waits=1):
    """walrus in this env encodes at most one sync-wait per instruction;
    split extras onto same-engine NoOps emitted just before."""
    for f in nc.m.functions:
        for bb in f.blocks:
            new = []
            for ins in bb.instructions:
                si = ins.sync_info
                conds = list(si.on_wait) if si is not None else []
                if len(conds) > max_waits:
                    for cond in conds[:-max_waits]:
                        n = mybir.InstNoOp(
                            name=nc.get_next_instruction_name(), ins=[], outs=[])
                        n.engine = ins.engine
                        n.sync_info = mybir.SyncInfo(on_wait=[cond], on_update=[])
                        new.append(n)
                    si.on_wait = conds[-max_waits:]
                new.append(ins)
            bb.instructions = new


def _build_nc(split=True):
    key = "nc" if split else "nc_nosplit"
    if key in _NC_CACHE:
        return _NC_CACHE[key]
    import concourse.bass as bass
    import concourse.tile as tile
    from concourse import mybir

    f32 = mybir.dt.float32
    bf16 = mybir.dt.bfloat16
    f8 = mybir.dt.float8e4
    AF = mybir.ActivationFunctionType
    MUL = mybir.AluOpType.mult
    ADD = mybir.AluOpType.add
    MIN = mybir.AluOpType.min
    DR = mybir.MatmulPerfMode.DoubleRow

    nc = bass.Bass()

    xT_d = nc.dram_tensor("xT", [C, T], bf16, kind="ExternalInput")
    wq_d = nc.dram_tensor("wqT", [C, NH * 64], bf16, kind="ExternalInput")
    wk_d = nc.dram_tensor("wkT", [C, 128], bf16, kind="ExternalInput")
    wv_d = nc.dram_tensor("wvT", [C, 128], bf16, kind="ExternalInput")
    wp_d = nc.dram_tensor("wpT", [NH * 64, C], bf16, kind="ExternalInput")
    wrep1_d = nc.dram_tensor("wrep1", [NH, WREP_W], bf16, kind="ExternalInput")
    u_d = nc.dram_tensor("usb", [128, NH], f32, kind="ExternalInput")
    bias_d = nc.dram_tensor("biassb", [128, NH], f32, kind="ExternalInput")
    out_d = nc.dram_tensor("out", [T, C], bf16, kind="ExternalOutput")

    xT_r = xT_d.rearrange("(k p) t -> p k t", p=128)
    wq_r = wq_d.rearrange("(k p) e -> p k e", p=128)

    with tile.TileContext(nc) as tc:
        with (
            tc.tile_pool(name="const", bufs=1) as const,
        ):
            # ---- persistent tiles ----
            wp = const.tile([128, 4, C], bf16)
            wrep = const.tile([128, NH, WREP_W], bf16)
            usb = const.tile([128, NH], f32)
            biassb = const.tile([128, NH], f32)

            kRep = const.tile([128, 2, T], bf16)     # kv on both halves
            qRep = const.tile([128, NH, T], bf16)    # slot i on both halves
            v_sb = const.tile([128, ST, 130], bf16)  # [s, j, (v_kv0|1|v_kv1|1)]
            outT = const.tile([128, 4, T], bf16)     # [(2 slots d), pair, t]
            dstack = const.tile([128, 128], bf16)    # [(slot,tt), t_in] denom
            rstb = const.tile([128, 128], bf16)      # 1/denom, bf16

            with (
                tc.tile_pool(name="ph1", bufs=1) as ph1,
                tc.tile_pool(name="work", bufs=3) as work,
                tc.tile_pool(name="ebuf", bufs=3) as ebufp,
                tc.tile_pool(name="dstgp", bufs=1) as dstgp,
                tc.tile_pool(name="dramd", bufs=1, space="DRAM") as dramd,
            ):
                xT = ph1.tile([128, KCT, T], bf16)
                wk = ph1.tile([128, KCT, 128], bf16)
                wq = ph1.tile([128, KCT, NH * 64], bf16)
                wv = ph1.tile([128, KCT, 128], bf16)
                ddrow = dramd.tile([NH, T], bf16)
                rdram = dramd.tile([NH, T], bf16)
                rd3 = rdram.rearrange("i (a b) -> i a b", b=128)

                # ---- DMA loads, ordered for the critical path ----
                # xT in column pieces: piece 0 (cols 0:512, all kc) unblocks
                # the first k/q projections ~15us before the full tensor.
                nc.sync.dma_start(out=wk, in_=wk_d.rearrange(
                    "(k p) e -> p k e", p=128))

                def wrep_bcast(i):
                    s = wrep1_d[i:i + 1, :]
                    s = bass.AP(tensor=s.tensor, offset=s.offset,
                                ap=[[0, 128]] + list(s.ap)[1:])
                    nc.gpsimd.dma_start(out=wrep[:, i, :], in_=s)

                nc.gpsimd.dma_start(out=usb, in_=u_d[:])
                nc.gpsimd.dma_start(out=biassb, in_=bias_d[:])
                wrep_bcast(_PORDER[0])
                qs = [nc.sync, nc.scalar, nc.gpsimd]
                for p4 in range(4):
                    sl4 = slice(512 * p4, 512 * (p4 + 1))
                    for kc in range(KCT):
                        qs[kc % 3].dma_start(out=xT[:, kc, sl4],
                                             in_=xT_r[:, kc, sl4])
                for kc in range(KCT):
                    nc.scalar.dma_start(out=wq[:, kc, :], in_=wq_r[:, kc, :])
                nc.gpsimd.dma_start(out=wv, in_=wv_d.rearrange(
                    "(k p) e -> p k e", p=128))
                for i in range(NH):
                    if i != _PORDER[0]:
                        wrep_bcast(i)
                nc.gpsimd.dma_start(out=wp, in_=wp_d.rearrange(
                    "(k p) e -> p k e", p=128))
                # p-state warm-up fodder (PE ramps to full clock after ~3us
                # of continuous work; keep it busy during the xT wait)
                warm = work.tile([128, 512], bf16, tag="warm", bufs=1)
                nc.vector.memset(warm, 0.0)
                # ones columns of v (disjoint from the v-proj copies)
                nc.vector.memset(v_sb[:, :, 64], 1.0)
                nc.vector.memset(v_sb[:, :, 129], 1.0)

                # ---- attention with interleaved projections/out-proj ----
                with (
                    tc.tile_pool(name="psA", bufs=1, space="PSUM") as psA,
                    tc.tile_pool(name="psS", bufs=2, space="PSUM") as psS,
                ):
                    def k_proj_chunk(sc):
                        """kRep cols [512sc,512sc+512): kv half0 rows 0:64,
                        half1 rows 64:128, then per-chunk duplication."""
                        ps = psS.tile([128, 512], f32, tag="S")
                        for kc in range(KCT):
                            nc.tensor.matmul(
                                ps, lhsT=wk[:, kc, :],
                                rhs=xT[:, kc, 512 * sc:512 * (sc + 1)],
                                start=(kc == 0), stop=(kc == KCT - 1))
                        sl = slice(512 * sc, 512 * (sc + 1))
                        nc.vector.tensor_copy(kRep[0:64, 0, sl], ps[0:64, :])
                        nc.vector.tensor_copy(kRep[64:128, 1, sl],
                                              ps[64:128, :])
                        nc.sync.dma_start(out=kRep[64:128, 0, sl],
                                          in_=kRep[0:64, 0, sl])
                        nc.sync.dma_start(out=kRep[0:64, 1, sl],
                                          in_=kRep[64:128, 1, sl])

                    def q_proj_block(p, tcn):
                        """qRep slots 2p/2p+1, cols [512tcn, 512tcn+512)."""
                        ps = psS.tile([128, 512], f32, tag="S")
                        for kc in range(KCT):
                            nc.tensor.matmul(
                                ps, lhsT=wq[:, kc, 128 * p:128 * (p + 1)],
                                rhs=xT[:, kc, 512 * tcn:512 * (tcn + 1)],
                                start=(kc == 0), stop=(kc == KCT - 1))
                        sl = slice(512 * tcn, 512 * (tcn + 1))
                        nc.vector.tensor_copy(qRep[0:64, 2 * p, sl],
                                              ps[0:64, :])
                        nc.vector.tensor_copy(qRep[64:128, 2 * p + 1, sl],
                                              ps[64:128, :])
                        nc.sync.dma_start(out=qRep[64:128, 2 * p, sl],
                                          in_=qRep[0:64, 2 * p, sl])
                        nc.sync.dma_start(out=qRep[0:64, 2 * p + 1, sl],
                                          in_=qRep[64:128, 2 * p + 1, sl])

                    def v_proj_block(st):
                        ps = psS.tile([128, 512], f32, tag="S")
                        for kc in range(KCT):
                            nc.tensor.matmul(
                                ps[:, 0:128],
                                lhsT=xT[:, kc, 128 * st:128 * (st + 1)],
                                rhs=wv[:, kc, :],
                                start=(kc == 0), stop=(kc == KCT - 1))
                        nc.vector.tensor_copy(v_sb[:, st, 0:64], ps[:, 0:64])
                        nc.vector.tensor_copy(v_sb[:, st, 65:129],
                                              ps[:, 64:128])

                    def pair_chain(p):
                        """denominator reciprocal + outT normalize, pair p."""
                        rows = slice(32 * p, 32 * (p + 1))
                        dstf = work.tile([32, 128], f32, tag="dstf")
                        nc.vector.tensor_copy(dstf, dstack[rows, :])
                        rstf = work.tile([32, 128], f32, tag="rstf")
                        nc.vector.reciprocal(rstf, dstf)
                        nc.vector.tensor_copy(rstb[rows, :], rstf)
                        for hh in range(2):
                            ii = 2 * p + hh
                            nc.sync.dma_start(
                                out=rd3[ii],
                                in_=rstb[16 * ii:16 * (ii + 1), :])
                        rrep = work.tile([128, T], bf16, tag="rrep", bufs=1)
                        for hh in range(2):
                            ii = 2 * p + hh
                            src = rdram[ii:ii + 1, :]
                            src = bass.AP(tensor=src.tensor, offset=src.offset,
                                          ap=[[0, 64]] + list(src.ap)[1:])
                            nc.sync.dma_start(
                                out=rrep[64 * hh:64 * hh + 64, :], in_=src)
                        nc.vector.tensor_tensor(outT[:, p, :], outT[:, p, :],
                                                rrep, MUL)

                    # prework gates the iteration's scores; midwork is
                    # emitted after the ACT issue (doesn't delay the exp).
                    prework = [[[] for _ in range(ST)] for _ in range(NH)]
                    midwork = [[[] for _ in range(ST)] for _ in range(NH)]
                    first_slot = _PORDER[0]
                    fq = first_slot // 2
                    # upfront: k chunk 0 + the first slot's gating q chunks
                    prework[0][0].append(lambda: k_proj_chunk(0))
                    ne0 = _N_EFF[first_slot][0]
                    for tcn in range(ne0):
                        prework[0][0].append(
                            lambda tcn=tcn: q_proj_block(fq, tcn))
                    for tcn in range(ne0, NCH):
                        jn = _J_FIRST[first_slot][tcn]
                        midwork[0][max(0, jn - 2)].insert(
                            0, lambda tcn=tcn: q_proj_block(fq, tcn))
                    for sc in (1, 2, 3):
                        midwork[0][4 * sc - 3].append(
                            lambda sc=sc: k_proj_chunk(sc))
                    for st in range(ST):
                        midwork[0][st].append(lambda st=st: v_proj_block(st))
                    # remaining q pairs at positions 1..3, j = 2,5,8,11;
                    # pair for the slots at positions 2m,2m+1 must be emitted
                    # by position 2m-1 -> emission order follows _PORDER.
                    qpairs = [_PORDER[2 * m] // 2 for m in range(1, 4)]
                    for pos, p in enumerate(qpairs):
                        for tcn in range(NCH):
                            midwork[1 + pos][2 + 3 * tcn].append(
                                lambda p=p, tcn=tcn: q_proj_block(p, tcn))

                    for _ in range(16):
                        pj = psA.tile([128, 512], f32, tag="pa")
                        nc.tensor.matmul(pj, lhsT=warm[:, 0:128],
                                         rhs=warm, start=True, stop=True)

                    for pos in range(NH):
                        i = _PORDER[pos]
                        p, half = i // 2, i % 2
                        pa = psA.tile([65, T], f32, tag="pa")
                        # diag mult min(exp(-a(t_in-127)), exp(a(127-s_in)))
                        dmin = work.tile([128, 128], bf16, tag="dmin")
                        nc.vector.tensor_scalar(dmin, wrep[:, i, 0:128],
                                                usb[:, i:i + 1], None, MIN)
                        for j in range(ST):
                            for fn in prework[pos][j]:
                                fn()
                            ne = _N_EFF[i][j]
                            W = 512 * ne
                            E = ebufp.tile([128, T], bf16, tag="E")
                            for sh in range(2):
                                c0, c1 = 2 * sh, min(ne, 2 * sh + 2)
                                if c0 >= c1:
                                    continue
                                S = psS.tile([128, 1024], f32, tag="S")
                                for tcn in range(c0, c1):
                                    rh = 64 * (tcn % 2)
                                    o = 512 * (tcn - c0)
                                    nc.tensor.matmul(
                                        S[:, o:o + 512],
                                        lhsT=kRep[rh:rh + 64, half,
                                                  128 * j:128 * (j + 1)],
                                        rhs=qRep[rh:rh + 64, i,
                                                 512 * tcn:512 * (tcn + 1)],
                                        start=True, stop=True)
                                wv_ = 512 * (c1 - c0)
                                nc.scalar.activation(
                                    E[:, 1024 * sh:1024 * sh + wv_],
                                    S[:, :wv_], AF.Exp,
                                    bias=biassb[:, i:i + 1], scale=0.125)
                            for fn in midwork[pos][j]:
                                fn()
                            lo = 128 * j       # t < lo: future region (mult u)
                            hi = 128 * (j + 1)  # t >= hi: past (Toeplitz)
                            if lo > 0:
                                nc.vector.tensor_scalar(
                                    E[:, :lo], E[:, :lo],
                                    usb[:, i:i + 1], None, MUL)
                            nc.vector.tensor_tensor(E[:, lo:hi], E[:, lo:hi],
                                                    dmin, MUL)
                            if W > hi:
                                nc.vector.tensor_tensor(
                                    E[:, hi:W], E[:, hi:W],
                                    wrep[:, i, 128:128 + (W - hi)], MUL)
                            for tcn in range(ne):
                                nc.tensor.matmul(
                                    pa[:, 512 * tcn:512 * (tcn + 1)],
                                    lhsT=v_sb[:, j, 65 * half:65 * half + 65],
                                    rhs=E[:, 512 * tcn:512 * (tcn + 1)],
                                    start=(j == _J_FIRST[i][tcn]),
                                    stop=(j == ST - 1),
                                    skip_group_check=True)
                        # copy-out: rows 0:64 -> outT half; row 64 -> denom
                        st65 = dstgp.tile([65, T], bf16, tag="st65")
                        nc.vector.tensor_copy(st65, pa[0:65, :])
                        nc.sync.dma_start(
                            out=outT[64 * half:64 * half + 64, p, :],
                            in_=st65[0:64, :])
                        nc.sync.dma_start(out=ddrow[i:i + 1, :],
                                          in_=st65[64:65, :])
                        nc.sync.dma_start(
                            out=dstack[16 * i:16 * (i + 1), :],
                            in_=ddrow[i].rearrange("(a b) -> a b", b=128))
                        if pos % 2 == 1:
                            pair_chain(p)

                    # ---- tail: output projection + output DMA ----
                    for tt in range(ST):
                        osb = work.tile([128, C], bf16, tag="osb", bufs=2)
                        for ec in range(2):
                            ps = psS.tile([128, 512], f32, tag="S")
                            for kt in range(4):
                                nc.tensor.matmul(
                                    ps,
                                    lhsT=outT[:, kt, 128 * tt:128 * (tt + 1)],
                                    rhs=wp[:, kt, 512 * ec:512 * (ec + 1)],
                                    start=(kt == 0), stop=(kt == 3))
                            nc.vector.tensor_copy(
                                osb[:, 512 * ec:512 * (ec + 1)], ps)
                        eng = nc.sync if tt % 2 == 0 else nc.scalar
                        eng.dma_start(out=out_d[128 * tt:128 * (tt + 1), :],
                                      in_=osb)

    if split:
        _split_multiwait(nc, mybir)
    _NC_CACHE[key] = nc
    return nc


def _prep_core_inputs(x, Wq, Wkv, Wproj, b, g):
    import ml_dtypes
    bf = ml_dtypes.bfloat16
    heads = [_head_of_slot(i, g) for i in range(NH)]
    xT = np.ascontiguousarray(x[b].T).astype(bf)                      # [C, T]
    wq_cols = np.concatenate([Wq[64 * h:64 * (h + 1)] for h in heads], axis=0)
    wqT = np.ascontiguousarray(wq_cols.T).astype(bf)                  # [C, 512]
    # kv heads used by parity-g slots: half 0 -> kv g, half 1 -> kv g+2
    kv = [g, g + 2]
    wk_rows = np.concatenate([Wkv[64 * kvh:64 * (kvh + 1)] for kvh in kv])
    wv_rows = np.concatenate([Wkv[256 + 64 * kvh:256 + 64 * (kvh + 1)]
                              for kvh in kv])
    wkT = np.ascontiguousarray(wk_rows.T).astype(bf)
    wvT = np.ascontiguousarray(wv_rows.T).astype(bf)
    cols = np.concatenate([np.arange(64 * h, 64 * (h + 1)) for h in heads])
    wpT = np.ascontiguousarray(Wproj[:, cols].T).astype(bf)           # [512, C]

    s_in = np.arange(128, dtype=np.float64)
    wrep1 = np.empty((NH, WREP_W), dtype=bf)
    u = np.empty((128, NH), dtype=np.float32)
    bias = np.empty((128, NH), dtype=np.float32)
    idx = np.arange(WREP_W, dtype=np.float64)
    for i, h in enumerate(heads):
        a = _a_of_head(h)
        wrep1[i] = np.exp(-a * (idx - 127.0)).astype(np.float32)
        u[:, i] = np.exp(a * (127.0 - s_in)).astype(np.float32)
        bias[:, i] = (a * (s_in - 127.0)).astype(np.float32)
    return {"xT": xT, "wqT": wqT, "wkT": wkT, "wvT": wvT, "wpT": wpT,
            "wrep1": wrep1, "usb": u, "biassb": bias}


def kernel(x, Wq, Wkv, Wproj, bproj):
    from concourse.bass_utils import run_bass_kernel_spmd
    x = np.asarray(x, dtype=np.float32)
    Wq = np.asarray(Wq, dtype=np.float32)
    Wkv = np.asarray(Wkv, dtype=np.float32)
    Wproj = np.asarray(Wproj, dtype=np.float32)
    bproj = np.asarray(bproj, dtype=np.float32)

    nc = _build_nc()
    in_maps = [_prep_core_inputs(x, Wq, Wkv, Wproj, c // 2, c % 2)
               for c in range(8)]
    res = run_bass_kernel_spmd(nc, in_maps, core_ids=list(range(8)))
    out = np.zeros((B, T, C), dtype=np.float32)
    for c in range(8):
        out[c // 2] += np.asarray(res.results[c]["out"], dtype=np.float32)
    out += bproj[None, None, :]
    return out
